# revision 13
# baseline (speedup 1.0000x reference)
"""PointNet feature interpolation (3-NN inverse-distance) Trainium2 kernel.

Problem (per batch b of 8, one NeuronCore each):
  xyz1:    [3, N=8192]   source point coords
  xyz2:    [3, S=2048]   query point coords
  points1: [D=256, N]    source features
  out:     [D, S]        interpolated features

Device algorithm per core (v5):
  1. negdist[s, n] = 2*x2_s.x1_n - |x2_s|^2 - |x1_n|^2 as one K=13 bf16
     matmul (hi/lo split products); fp32 PSUM, drained to fp16 by ScalarE.
  2. Pairwise tensor-max tree (DVE fp16 2x) folds 8192 -> 512 block maxima
     (block j = points {j + 512m}); max8/max_index give top-4 blocks/row
     (exact top-3 blocks are guaranteed to contain the top-3 points; the
     4th is slack for fp16 rounding ties).
  3. One dma_gather per 4-chunk group fetches the blocks' [x,y,z,pad]
     rows; candidate d^2 is recomputed exactly in fp32; a 7-bit candidate
     slot id is OR-ed into the low mantissa bits of -d^2 so one max8
     yields top-3 values and slots; slot -> global index via a 4-wide
     equality match against the block ids (no gather needed).
  4. weights w_k = (1/(d_k+1e-8))/sum; one dma_gather per group fetches
     the 3 neighbor feature rows (fp16); weighted sum; PE-transpose.

dma_gather index images are built via a DRAM bounce (the wrapped
[i%16, i//16] layout can't be produced by an SBUF-side DMA since DMA
cannot split an SBUF partition dim) and replicated to the 8 partition
groups by log-doubling SBUF-to-SBUF copies.
"""

import numpy as np
import ml_dtypes

B, N, S, D = 8, 8192, 2048, 256
P = 128
NCHUNK = S // P      # 16 query-row chunks per core
CP = 4               # chunks per group (back-end batch)
NGRP = NCHUNK // CP
NT = 512             # matmul moving free dim (one PSUM bank)
NNT = N // NT        # 16
K = 13               # contraction rows of the distance matmul
NBLK = 512           # block-maxima per row (block = 16 points stride 512)
BPTS = N // NBLK     # 16 points per block
NB = 4               # candidate blocks kept per row
NCAND = NB * BPTS    # 64 candidate points per row

QMASK = 0xFFFFFF80   # keep sign+exp+16 mantissa bits of -d^2
SMASK = 0x0000007F   # low 7 bits: candidate slot (k*16+m)
SIGN = 0x80000000

_COMPILED = None


def _build_bass(reps=1, abl=()):
    import concourse.bass as bass
    import concourse.mybir as mybir
    import concourse.tile as tile
    from concourse import bacc
    from concourse.masks import make_identity

    f32 = mybir.dt.float32
    f16 = mybir.dt.float16
    u32 = mybir.dt.uint32
    u16 = mybir.dt.uint16
    i16 = mybir.dt.int16
    Alu = mybir.AluOpType
    X = mybir.AxisListType.X

    nc = bacc.Bacc(None)
    x2m = nc.dram_tensor("x2m", [K, S], mybir.dt.bfloat16, kind="ExternalInput")
    x1m = nc.dram_tensor("x1m", [K, N], mybir.dt.bfloat16, kind="ExternalInput")
    p1t = nc.dram_tensor("p1t", [N, D], f16, kind="ExternalInput")
    xblk = nc.dram_tensor("xblk", [NBLK, BPTS * 4], f32, kind="ExternalInput")
    x2n = nc.dram_tensor("x2n", [P, NCHUNK, 4], f32, kind="ExternalInput")
    outT = nc.dram_tensor("outT", [D, S], f32, kind="ExternalOutput")

    def wrap_bounce(name, src_sb, nslot, dram_pool, sb_pool):
        """src_sb: [P, nslot] u16 per-partition indices ->
        wrapped [P, nslot*8] u16 image for dma_gather (idx i at
        [i%16, i//16], replicated to the 8 partition groups)."""
        dimg = dram_pool.tile([P, nslot], u16, tag=name + "_d")
        nc.sync.dma_start(dimg[:], src_sb)
        w = sb_pool.tile([P, nslot * 8], u16, tag=name + "_w")
        nc.sync.dma_start(
            w[0:16, :].rearrange("q (c j) -> q c j", j=8),
            dimg[:].rearrange("(j q) c -> q c j", q=16))
        nc.sync.dma_start(w[16:32, :], w[0:16, :])
        nc.sync.dma_start(w[32:64, :], w[0:32, :])
        nc.sync.dma_start(w[64:128, :], w[0:64, :])
        return w

    with tile.TileContext(nc) as tc:
        with (
            tc.tile_pool(name="const", bufs=1) as cpool,
            tc.tile_pool(name="negb", bufs=2) as nbpool,
            tc.tile_pool(name="tree", bufs=2) as tpool,
            tc.tile_pool(name="mm", bufs=2, space="PSUM") as mmpool,
            tc.tile_pool(name="small", bufs=2) as spool,
            tc.tile_pool(name="gat", bufs=2) as gpool,
            tc.tile_pool(name="oput", bufs=3) as opool,
            tc.tile_pool(name="dscr", bufs=2, space="DRAM") as dpool,
        ):
            x2s = cpool.tile([K, S], mybir.dt.bfloat16)
            nc.sync.dma_start(x2s[:], x2m[:])
            x1s = cpool.tile([K, N], mybir.dt.bfloat16)
            nc.sync.dma_start(x1s[:], x1m[:])
            x2n_sb = cpool.tile([P, NCHUNK, 4], f32)
            nc.sync.dma_start(x2n_sb[:], x2n[:])
            ident = cpool.tile([P, P], f16)
            make_identity(nc, ident[:])
            # slot id table: slots[p, k, m] = k*16 + m
            slots = cpool.tile([P, NB, BPTS], u32)
            nc.gpsimd.iota(slots[:], pattern=[[BPTS, NB], [1, BPTS]],
                           base=0, channel_multiplier=0)
            # block id iota: kio[p, k] = k
            kio = cpool.tile([P, NB], u32)
            nc.gpsimd.iota(kio[:], pattern=[[1, NB]], base=0,
                           channel_multiplier=0)

            for _rep in range(reps):
                for gg in range(NGRP):
                    bi32 = spool.tile([P, CP, NB], u32, tag="bi32")
                    # ---- pass 1 per chunk: distances, block maxima ----
                    for cc in range(CP):
                        ci = gg * CP + cc
                        negb = nbpool.tile([P, N], f16)
                        for q in range(4):
                            ps = mmpool.tile([P, 2048], f32, tag="mm")
                            for j in range(4):
                                nt = q * 4 + j
                                nc.tensor.matmul(
                                    ps[:, j * NT:(j + 1) * NT],
                                    lhsT=x2s[:, ci * P:(ci + 1) * P],
                                    rhs=x1s[:, nt * NT:(nt + 1) * NT],
                                    start=True,
                                    stop=True,
                                )
                            nc.scalar.copy(negb[:, q * 2048:(q + 1) * 2048],
                                           ps[:])

                        # pairwise-max tree 8192 -> 512 (fp16 2x mode)
                        bm = tpool.tile([P, NBLK], f16, tag="bm")
                        tsc = tpool.tile([P, 4096], f16, tag="tree")
                        nc.vector.tensor_tensor(
                            out=tsc[:], in0=negb[:, 0:4096],
                            in1=negb[:, 4096:8192], op=Alu.max)
                        nc.vector.tensor_tensor(
                            out=tsc[:, 0:2048], in0=tsc[:, 0:2048],
                            in1=tsc[:, 2048:4096], op=Alu.max)
                        nc.vector.tensor_tensor(
                            out=tsc[:, 0:1024], in0=tsc[:, 0:1024],
                            in1=tsc[:, 1024:2048], op=Alu.max)
                        nc.vector.tensor_tensor(
                            out=bm[:], in0=tsc[:, 0:512],
                            in1=tsc[:, 512:1024], op=Alu.max)

                        bv8 = spool.tile([P, 8], f16, tag="bv8")
                        nc.vector.max(out=bv8[:], in_=bm[:])
                        i8 = spool.tile([P, 8], u32, tag="i8")
                        nc.vector.max_index(out=i8[:], in_max=bv8[:],
                                            in_values=bm[:])
                        nc.vector.tensor_copy(bi32[:, cc, :], i8[:, 0:NB])

                    # ---- gather the group's blocks in one dma_gather ----
                    bi16 = spool.tile([P, CP * NB], u16, tag="bi16")
                    nc.vector.tensor_copy(
                        bi16[:], bi32[:].rearrange("p c n -> p (c n)"))
                    wbi = wrap_bounce("bi", bi16[:], CP * NB, dpool, spool)
                    gxb = gpool.tile([P, CP, NB, BPTS * 4], f32, tag="gxb")
                    nc.gpsimd.dma_gather(
                        out_ap=gxb[:].rearrange("p c n f -> p (c n) f"),
                        in_ap=xblk[:],
                        idxs_ap=wbi[:].bitcast(i16),
                        num_idxs=CP * NB * P,
                        num_idxs_reg=CP * NB * P,
                        elem_size=BPTS * 4,
                        single_packet=False,
                    )

                    # ---- exact fp32 candidate distances ----
                    gv = gxb[:].rearrange("p c k (m f) -> p c k m f", f=4)
                    sh = [P, CP, NB, BPTS]

                    def x2c(c):
                        return (x2n_sb[:, gg * CP:(gg + 1) * CP, c:c + 1]
                                .unsqueeze(3).to_broadcast(sh))

                    dx = spool.tile(sh, f32, tag="dx")
                    d2 = spool.tile(sh, f32, tag="d2")
                    tmp = spool.tile(sh, f32, tag="tmpA")
                    nc.vector.tensor_tensor(out=dx[:], in0=gv[:, :, :, :, 0],
                                            in1=x2c(0), op=Alu.subtract)
                    nc.vector.tensor_tensor(out=d2[:], in0=dx[:], in1=dx[:],
                                            op=Alu.mult)
                    nc.vector.tensor_tensor(out=dx[:], in0=gv[:, :, :, :, 1],
                                            in1=x2c(1), op=Alu.subtract)
                    nc.vector.tensor_tensor(out=tmp[:], in0=dx[:], in1=dx[:],
                                            op=Alu.mult)
                    nc.gpsimd.tensor_tensor(out=d2[:], in0=d2[:], in1=tmp[:],
                                            op=Alu.add)
                    nc.vector.tensor_tensor(out=dx[:], in0=gv[:, :, :, :, 2],
                                            in1=x2c(2), op=Alu.subtract)
                    nc.vector.tensor_tensor(out=tmp[:], in0=dx[:], in1=dx[:],
                                            op=Alu.mult)
                    nc.gpsimd.tensor_tensor(out=d2[:], in0=d2[:], in1=tmp[:],
                                            op=Alu.add)

                    # ---- slot-in-mantissa decode ----
                    # vq = (-d2 with low 7 bits cleared) | slot
                    vq = spool.tile(sh, u32, tag="vq")
                    nc.vector.tensor_scalar(
                        out=vq[:], in0=d2[:].bitcast(u32), scalar1=QMASK,
                        scalar2=SIGN, op0=Alu.bitwise_and, op1=Alu.bitwise_xor)
                    nc.vector.tensor_tensor(
                        out=vq[:], in0=vq[:],
                        in1=slots[:].unsqueeze(1).to_broadcast(sh),
                        op=Alu.bitwise_or)
                    m8 = spool.tile([P, CP, 8], f32, tag="m8")
                    for cc in range(CP):
                        nc.vector.max(out=m8[:, cc, :],
                                      in_=vq[:, cc].bitcast(f32))
                    m8u = m8[:].bitcast(u32)

                    # slot -> (k, m); global index n3 = bi[k] | (m << 9)
                    sh3 = [P, CP, 3]
                    sh34 = [P, CP, 3, NB]
                    s3 = spool.tile(sh3, u32, tag="s3")
                    nc.vector.tensor_scalar(out=s3[:], in0=m8u[:, :, 0:3],
                                            scalar1=SMASK, scalar2=None,
                                            op0=Alu.bitwise_and)
                    k3 = spool.tile(sh3, u32, tag="k3")
                    nc.vector.tensor_scalar(out=k3[:], in0=s3[:],
                                            scalar1=4, scalar2=None,
                                            op0=Alu.logical_shift_right)
                    m3 = spool.tile(sh3, u32, tag="m3")
                    nc.vector.tensor_scalar(out=m3[:], in0=s3[:],
                                            scalar1=0xF, scalar2=9,
                                            op0=Alu.bitwise_and,
                                            op1=Alu.logical_shift_left)
                    eq = spool.tile(sh34, u32, tag="eq")
                    nc.vector.tensor_tensor(
                        out=eq[:],
                        in0=k3[:].unsqueeze(3).to_broadcast(sh34),
                        in1=kio[:].unsqueeze(1).unsqueeze(2)
                        .to_broadcast(sh34),
                        op=Alu.is_equal)
                    nc.vector.tensor_tensor(
                        out=eq[:], in0=eq[:],
                        in1=bi32[:].unsqueeze(2).to_broadcast(sh34),
                        op=Alu.mult)
                    bsel = spool.tile(sh3, u32, tag="bsel")
                    nc.vector.tensor_reduce(out=bsel[:], in_=eq[:], axis=X,
                                            op=Alu.max)
                    n3_32 = spool.tile(sh3, u32, tag="n3_32")
                    nc.vector.tensor_tensor(out=n3_32[:], in0=bsel[:],
                                            in1=m3[:], op=Alu.bitwise_or)
                    n3_16 = spool.tile([P, CP * 3], u16, tag="n3_16")
                    nc.vector.tensor_copy(
                        n3_16[:], n3_32[:].rearrange("p c n -> p (c n)"))

                    # ---- weights ----
                    d3 = spool.tile(sh3, f32, tag="d3")
                    nc.vector.tensor_scalar(out=d3[:].bitcast(u32),
                                            in0=m8u[:, :, 0:3],
                                            scalar1=QMASK, scalar2=SIGN,
                                            op0=Alu.bitwise_and,
                                            op1=Alu.bitwise_xor)
                    nc.vector.tensor_scalar(out=d3[:], in0=d3[:],
                                            scalar1=1e-8, scalar2=None,
                                            op0=Alu.add)
                    nc.vector.reciprocal(d3[:], d3[:])
                    rsum = spool.tile([P, CP], f32, tag="rsum")
                    nc.vector.tensor_reduce(out=rsum[:], in_=d3[:], axis=X,
                                            op=Alu.add)
                    nc.vector.reciprocal(rsum[:], rsum[:])
                    w3 = spool.tile(sh3, f32, tag="w3")
                    nc.vector.tensor_tensor(
                        out=w3[:], in0=d3[:],
                        in1=rsum[:].unsqueeze(2).to_broadcast(sh3),
                        op=Alu.mult)

                    # ---- feature gather (one dma_gather per group) ----
                    wn3 = wrap_bounce("n3", n3_16[:], CP * 3, dpool, spool)
                    g = gpool.tile([P, CP, 3, D], f16, tag="g")
                    nc.gpsimd.dma_gather(
                        out_ap=g[:].rearrange("p c k f -> p (c k) f"),
                        in_ap=p1t[:],
                        idxs_ap=wn3[:].bitcast(i16),
                        num_idxs=CP * 3 * P,
                        num_idxs_reg=CP * 3 * P,
                        elem_size=D,
                        single_packet=False,
                    )

                    # ---- interpolate + transpose out ----
                    for cc in range(CP):
                        ci = gg * CP + cc
                        acc = opool.tile([P, D], f16, tag="acc")
                        nc.vector.tensor_scalar(
                            out=acc[:], in0=g[:, cc, 0, :],
                            scalar1=w3[:, cc, 0:1], scalar2=None,
                            op0=Alu.mult)
                        for k in (1, 2):
                            gm = opool.tile([P, D], f16, tag="gm")
                            nc.vector.tensor_scalar(
                                out=gm[:], in0=g[:, cc, k, :],
                                scalar1=w3[:, cc, k:k + 1], scalar2=None,
                                op0=Alu.mult)
                            nc.vector.tensor_tensor(out=acc[:], in0=acc[:],
                                                    in1=gm[:], op=Alu.add)

                        for dh in range(2):
                            pt = mmpool.tile([P, P], f16, tag="mm")
                            nc.tensor.transpose(
                                pt[:], acc[:, dh * P:(dh + 1) * P], ident[:])
                            ot = opool.tile([P, P], f32, tag="ot")
                            nc.scalar.copy(ot[:], pt[:])
                            nc.sync.dma_start(
                                outT[dh * P:(dh + 1) * P, ci * P:(ci + 1) * P],
                                ot[:])

    nc.finalize()
    return nc


def _split2(x):
    """Split fp64 array into 2 bf16 terms h+l ~ x (residual ~2^-17|x|)."""
    bf = ml_dtypes.bfloat16
    h = x.astype(bf)
    r = x - h.astype(np.float64)
    l = r.astype(bf)
    return h, l


def _host_matrices(xyz2b, xyz1b):
    """Build the K=13 bf16 contraction matrices for one batch.

    negdist[s, n] = sum_k X2[k, s] * X1[k, n]
                  = 2 * x2_s . x1_n - |x2_s|^2 - |x1_n|^2   (+O(2^-16))
    """
    bf = ml_dtypes.bfloat16
    x2 = xyz2b.astype(np.float64)   # [3, S]
    x1 = xyz1b.astype(np.float64)   # [3, N]
    n2 = (x2 * x2).sum(axis=0)      # [S]
    n1 = (x1 * x1).sum(axis=0)      # [N]

    Srows, Nrows = [], []
    for c in range(3):
        h2, l2 = _split2(x2[c])
        h1, l1 = _split2(x1[c])
        th2 = (2.0 * h2.astype(np.float64)).astype(bf)
        tl2 = (2.0 * l2.astype(np.float64)).astype(bf)
        # products kept: hh hl lh  (ll and smaller dropped)
        for a, b_ in ((th2, h1), (th2, l1), (tl2, h1)):
            Srows.append(a)
            Nrows.append(b_)
    ones_s = np.ones(x2.shape[1], dtype=bf)
    ones_n = np.ones(x1.shape[1], dtype=bf)
    for t in _split2(-n2):
        Srows.append(t)
        Nrows.append(ones_n)
    for t in _split2(-n1):
        Srows.append(ones_s)
        Nrows.append(t)
    X2 = np.stack([np.asarray(r, dtype=bf) for r in Srows])   # [13, S]
    X1 = np.stack([np.asarray(r, dtype=bf) for r in Nrows])   # [13, N]
    return X2, X1, n2.astype(np.float32), n1.astype(np.float32)


def _prep_inputs(xyz1, xyz2, points1):
    f16 = np.float16
    xyz1 = np.asarray(xyz1, dtype=np.float32)
    xyz2 = np.asarray(xyz2, dtype=np.float32)
    points1 = np.asarray(points1, dtype=np.float32)
    in_maps = []
    for b in range(B):
        X2, X1, n2, n1 = _host_matrices(xyz2[b], xyz1[b])
        p1tb = np.ascontiguousarray(points1[b].T).astype(f16)  # [N, D] fp16
        # block table: row j holds points {j + 512*m}, each [x, y, z, 0]
        xb = np.zeros((NBLK, BPTS, 4), dtype=np.float32)
        pts = xyz1[b].T.reshape(BPTS, NBLK, 3)     # [m, j, 3]
        xb[:, :, 0:3] = pts.transpose(1, 0, 2)
        # per-query [x, y, z, n2], laid out [p, chunk, 4]
        xq = np.empty((P, NCHUNK, 4), dtype=np.float32)
        q = xyz2[b].T.reshape(NCHUNK, P, 3)        # [chunk, p, 3]
        xq[:, :, 0:3] = q.transpose(1, 0, 2)
        xq[:, :, 3] = n2.reshape(NCHUNK, P).T
        in_maps.append({
            "x2m": X2, "x1m": X1, "p1t": p1tb,
            "xblk": xb.reshape(NBLK, BPTS * 4), "x2n": xq,
        })
    return in_maps


def _get_compiled():
    global _COMPILED
    if _COMPILED is None:
        _COMPILED = _build_bass()
    return _COMPILED


def kernel(xyz1, xyz2, points1):
    from concourse.bass_utils import run_bass_kernel_spmd

    nc = _get_compiled()
    in_maps = _prep_inputs(xyz1, xyz2, points1)
    res = run_bass_kernel_spmd(nc, in_maps, core_ids=list(range(B)))
    return np.stack([r["outT"] for r in res.results]).astype(np.float32)


if __name__ == "__main__":
    rng = np.random.default_rng(0)
    xyz1 = rng.standard_normal((B, 3, N), dtype=np.float32)
    xyz2 = rng.standard_normal((B, 3, S), dtype=np.float32)
    p1 = rng.standard_normal((B, D, N), dtype=np.float32)
    out = kernel(xyz1, xyz2, p1)
    print("out", out.shape, out.dtype)


# revision 16
# speedup vs baseline: 1.3706x; 1.3706x over previous
"""PointNet feature interpolation (3-NN inverse-distance) Trainium2 kernel.

Problem (per batch b of 8, one NeuronCore each):
  xyz1:    [3, N=8192]   source point coords
  xyz2:    [3, S=2048]   query point coords
  points1: [D=256, N]    source features
  out:     [D, S]        interpolated features

Device algorithm per core (v5):
  1. negdist[s, n] = 2*x2_s.x1_n - |x2_s|^2 - |x1_n|^2 as one K=13 bf16
     matmul (hi/lo split products); fp32 PSUM, drained to fp16 by ScalarE.
  2. Pairwise tensor-max tree (DVE fp16 2x) folds 8192 -> 512 block maxima
     (block j = points {j + 512m}); max8/max_index give top-4 blocks/row
     (exact top-3 blocks are guaranteed to contain the top-3 points; the
     4th is slack for fp16 rounding ties).
  3. One dma_gather per 4-chunk group fetches the blocks' [x,y,z,pad]
     rows; candidate d^2 is recomputed exactly in fp32; a 7-bit candidate
     slot id is OR-ed into the low mantissa bits of -d^2 so one max8
     yields top-3 values and slots; slot -> global index via a 4-wide
     equality match against the block ids (no gather needed).
  4. weights w_k = (1/(d_k+1e-8))/sum; one dma_gather per group fetches
     the 3 neighbor feature rows (fp16); weighted sum; PE-transpose.

dma_gather index images are built via a DRAM bounce (the wrapped
[i%16, i//16] layout can't be produced by an SBUF-side DMA since DMA
cannot split an SBUF partition dim) and replicated to the 8 partition
groups by log-doubling SBUF-to-SBUF copies.
"""

import numpy as np
import ml_dtypes

B, N, S, D = 8, 8192, 2048, 256
P = 128
NCHUNK = S // P      # 16 query-row chunks per core
CP = 4               # chunks per group (back-end batch)
NGRP = NCHUNK // CP
NT = 512             # matmul moving free dim (one PSUM bank)
NNT = N // NT        # 16
K = 13               # contraction rows of the distance matmul
NBLK = 512           # block-maxima per row (block = 16 points stride 512)
BPTS = N // NBLK     # 16 points per block
NB = 4               # candidate blocks kept per row
NCAND = NB * BPTS    # 64 candidate points per row

QMASK = 0xFFFFFF80   # keep sign+exp+16 mantissa bits of -d^2
SMASK = 0x0000007F   # low 7 bits: candidate slot (k*16+m)
SIGN = 0x80000000

_COMPILED = None


def _build_bass(reps=1, abl=()):
    import concourse.bass as bass
    import concourse.mybir as mybir
    import concourse.tile as tile
    from concourse import bacc
    from concourse.masks import make_identity

    f32 = mybir.dt.float32
    f16 = mybir.dt.float16
    u32 = mybir.dt.uint32
    u16 = mybir.dt.uint16
    i16 = mybir.dt.int16
    Alu = mybir.AluOpType
    X = mybir.AxisListType.X

    nc = bacc.Bacc(None)
    x2m = nc.dram_tensor("x2m", [K, S], mybir.dt.bfloat16, kind="ExternalInput")
    x1m = nc.dram_tensor("x1m", [K, N], mybir.dt.bfloat16, kind="ExternalInput")
    p1t = nc.dram_tensor("p1t", [N, D], f16, kind="ExternalInput")
    xblk = nc.dram_tensor("xblk", [NBLK, BPTS * 4], f32, kind="ExternalInput")
    x2n = nc.dram_tensor("x2n", [P, NCHUNK, 4], f32, kind="ExternalInput")
    outT = nc.dram_tensor("outT", [D, S], f32, kind="ExternalOutput")

    def wrap_bounce(name, src_sb, nslot, dram_pool, sb_pool):
        """src_sb: [P, nslot] u16 per-partition indices ->
        wrapped [P, nslot*8] u16 image for dma_gather (idx i at
        [i%16, i//16], replicated to the 8 partition groups)."""
        dimg = dram_pool.tile([P, nslot], u16, tag=name + "_d")
        nc.sync.dma_start(dimg[:], src_sb)
        w = sb_pool.tile([P, nslot * 8], u16, tag=name + "_w")
        nc.sync.dma_start(
            w[0:16, :].rearrange("q (c j) -> q c j", j=8),
            dimg[:].rearrange("(j q) c -> q c j", q=16))
        nc.sync.dma_start(w[16:32, :], w[0:16, :])
        nc.sync.dma_start(w[32:64, :], w[0:32, :])
        nc.sync.dma_start(w[64:128, :], w[0:64, :])
        return w

    with tile.TileContext(nc) as tc:
        with (
            tc.tile_pool(name="const", bufs=1) as cpool,
            tc.tile_pool(name="negb", bufs=2) as nbpool,
            tc.tile_pool(name="tree", bufs=2) as tpool,
            tc.tile_pool(name="mm", bufs=2, space="PSUM") as mmpool,
            tc.tile_pool(name="tp", bufs=2, space="PSUM") as tppool,
            tc.tile_pool(name="small", bufs=2) as spool,
            tc.tile_pool(name="gat", bufs=2) as gpool,
            tc.tile_pool(name="oput", bufs=3) as opool,
            tc.tile_pool(name="dscr", bufs=2, space="DRAM") as dpool,
        ):
            x2s = cpool.tile([K, S], mybir.dt.bfloat16)
            nc.sync.dma_start(x2s[:], x2m[:])
            x1s = cpool.tile([K, N], mybir.dt.bfloat16)
            nc.sync.dma_start(x1s[:], x1m[:])
            x2n_sb = cpool.tile([P, NCHUNK, 4], f32)
            nc.sync.dma_start(x2n_sb[:], x2n[:])
            ident = cpool.tile([P, P], f16)
            make_identity(nc, ident[:])
            # slot id table: slots[p, k, m] = k*16 + m
            slots = cpool.tile([P, NB, BPTS], u32)
            nc.gpsimd.iota(slots[:], pattern=[[BPTS, NB], [1, BPTS]],
                           base=0, channel_multiplier=0)
            # block id iota: kio[p, k] = k
            kio = cpool.tile([P, NB], u32)
            nc.gpsimd.iota(kio[:], pattern=[[1, NB]], base=0,
                           channel_multiplier=0)

            for _rep in range(reps):
                for gg in range(NGRP):
                    bi32 = spool.tile([P, CP, NB], u32, tag="bi32")
                    # ---- pass 1 per chunk: distances, block maxima ----
                    for cc in range(CP):
                        ci = gg * CP + cc
                        negb = nbpool.tile([P, N], f16)
                        # 16 NT segments in PSUM tiles of 3+3+3+3+2+2
                        # (3-bank mm tiles x2 bufs + 2-bank transpose pool
                        # fit the 8 PSUM banks without cross-group coupling)
                        nt0 = 0
                        for nseg in (3, 3, 3, 3, 2, 2):
                            ps = mmpool.tile([P, nseg * NT], f32, tag="mm")
                            for j in range(nseg):
                                nt = nt0 + j
                                nc.tensor.matmul(
                                    ps[:, j * NT:(j + 1) * NT],
                                    lhsT=x2s[:, ci * P:(ci + 1) * P],
                                    rhs=x1s[:, nt * NT:(nt + 1) * NT],
                                    start=True,
                                    stop=True,
                                )
                            nc.scalar.copy(
                                negb[:, nt0 * NT:(nt0 + nseg) * NT], ps[:])
                            nt0 += nseg

                        # pairwise-max tree 8192 -> 512 (fp16 2x mode)
                        bm = tpool.tile([P, NBLK], f16, tag="bm")
                        tsc = tpool.tile([P, 4096], f16, tag="tree")
                        nc.vector.tensor_tensor(
                            out=tsc[:], in0=negb[:, 0:4096],
                            in1=negb[:, 4096:8192], op=Alu.max)
                        nc.vector.tensor_tensor(
                            out=tsc[:, 0:2048], in0=tsc[:, 0:2048],
                            in1=tsc[:, 2048:4096], op=Alu.max)
                        nc.vector.tensor_tensor(
                            out=tsc[:, 0:1024], in0=tsc[:, 0:1024],
                            in1=tsc[:, 1024:2048], op=Alu.max)
                        nc.vector.tensor_tensor(
                            out=bm[:], in0=tsc[:, 0:512],
                            in1=tsc[:, 512:1024], op=Alu.max)

                        bv8 = spool.tile([P, 8], f16, tag="bv8")
                        nc.vector.max(out=bv8[:], in_=bm[:])
                        i8 = spool.tile([P, 8], u32, tag="i8")
                        nc.vector.max_index(out=i8[:], in_max=bv8[:],
                                            in_values=bm[:])
                        nc.vector.tensor_copy(bi32[:, cc, :], i8[:, 0:NB])

                    # ---- gather the group's blocks in one dma_gather ----
                    bi16 = spool.tile([P, CP * NB], u16, tag="bi16")
                    nc.vector.tensor_copy(
                        bi16[:], bi32[:].rearrange("p c n -> p (c n)"))
                    wbi = wrap_bounce("bi", bi16[:], CP * NB, dpool, spool)
                    gxb = gpool.tile([P, CP, NB, BPTS * 4], f32, tag="gxb")
                    nc.gpsimd.dma_gather(
                        out_ap=gxb[:].rearrange("p c n f -> p (c n) f"),
                        in_ap=xblk[:],
                        idxs_ap=wbi[:].bitcast(i16),
                        num_idxs=CP * NB * P,
                        num_idxs_reg=CP * NB * P,
                        elem_size=BPTS * 4,
                        single_packet=False,
                    )

                    # ---- exact fp32 candidate distances ----
                    gv = gxb[:].rearrange("p c k (m f) -> p c k m f", f=4)
                    sh = [P, CP, NB, BPTS]

                    def x2c(c):
                        return (x2n_sb[:, gg * CP:(gg + 1) * CP, c:c + 1]
                                .unsqueeze(3).to_broadcast(sh))

                    dx = spool.tile(sh, f32, tag="dx")
                    d2 = spool.tile(sh, f32, tag="d2")
                    tmp = spool.tile(sh, f32, tag="tmpA")
                    nc.vector.tensor_tensor(out=dx[:], in0=gv[:, :, :, :, 0],
                                            in1=x2c(0), op=Alu.subtract)
                    nc.vector.tensor_tensor(out=d2[:], in0=dx[:], in1=dx[:],
                                            op=Alu.mult)
                    nc.vector.tensor_tensor(out=dx[:], in0=gv[:, :, :, :, 1],
                                            in1=x2c(1), op=Alu.subtract)
                    nc.vector.tensor_tensor(out=tmp[:], in0=dx[:], in1=dx[:],
                                            op=Alu.mult)
                    nc.gpsimd.tensor_tensor(out=d2[:], in0=d2[:], in1=tmp[:],
                                            op=Alu.add)
                    nc.vector.tensor_tensor(out=dx[:], in0=gv[:, :, :, :, 2],
                                            in1=x2c(2), op=Alu.subtract)
                    nc.vector.tensor_tensor(out=tmp[:], in0=dx[:], in1=dx[:],
                                            op=Alu.mult)
                    nc.gpsimd.tensor_tensor(out=d2[:], in0=d2[:], in1=tmp[:],
                                            op=Alu.add)

                    # ---- slot-in-mantissa decode ----
                    # vq = (-d2 with low 7 bits cleared) | slot
                    vq = spool.tile(sh, u32, tag="vq")
                    nc.vector.tensor_scalar(
                        out=vq[:], in0=d2[:].bitcast(u32), scalar1=QMASK,
                        scalar2=SIGN, op0=Alu.bitwise_and, op1=Alu.bitwise_xor)
                    nc.vector.tensor_tensor(
                        out=vq[:], in0=vq[:],
                        in1=slots[:].unsqueeze(1).to_broadcast(sh),
                        op=Alu.bitwise_or)
                    m8 = spool.tile([P, CP, 8], f32, tag="m8")
                    for cc in range(CP):
                        nc.vector.max(out=m8[:, cc, :],
                                      in_=vq[:, cc].bitcast(f32))
                    m8u = m8[:].bitcast(u32)

                    # slot -> (k, m); global index n3 = bi[k] | (m << 9)
                    sh3 = [P, CP, 3]
                    sh34 = [P, CP, 3, NB]
                    s3 = spool.tile(sh3, u32, tag="s3")
                    nc.vector.tensor_scalar(out=s3[:], in0=m8u[:, :, 0:3],
                                            scalar1=SMASK, scalar2=None,
                                            op0=Alu.bitwise_and)
                    k3 = spool.tile(sh3, u32, tag="k3")
                    nc.vector.tensor_scalar(out=k3[:], in0=s3[:],
                                            scalar1=4, scalar2=None,
                                            op0=Alu.logical_shift_right)
                    m3 = spool.tile(sh3, u32, tag="m3")
                    nc.vector.tensor_scalar(out=m3[:], in0=s3[:],
                                            scalar1=0xF, scalar2=9,
                                            op0=Alu.bitwise_and,
                                            op1=Alu.logical_shift_left)
                    eq = spool.tile(sh34, u32, tag="eq")
                    nc.vector.tensor_tensor(
                        out=eq[:],
                        in0=k3[:].unsqueeze(3).to_broadcast(sh34),
                        in1=kio[:].unsqueeze(1).unsqueeze(2)
                        .to_broadcast(sh34),
                        op=Alu.is_equal)
                    nc.vector.tensor_tensor(
                        out=eq[:], in0=eq[:],
                        in1=bi32[:].unsqueeze(2).to_broadcast(sh34),
                        op=Alu.mult)
                    bsel = spool.tile(sh3, u32, tag="bsel")
                    nc.vector.tensor_reduce(out=bsel[:], in_=eq[:], axis=X,
                                            op=Alu.max)
                    n3_32 = spool.tile(sh3, u32, tag="n3_32")
                    nc.vector.tensor_tensor(out=n3_32[:], in0=bsel[:],
                                            in1=m3[:], op=Alu.bitwise_or)
                    n3_16 = spool.tile([P, CP * 3], u16, tag="n3_16")
                    nc.vector.tensor_copy(
                        n3_16[:], n3_32[:].rearrange("p c n -> p (c n)"))

                    # ---- weights ----
                    d3 = spool.tile(sh3, f32, tag="d3")
                    nc.vector.tensor_scalar(out=d3[:].bitcast(u32),
                                            in0=m8u[:, :, 0:3],
                                            scalar1=QMASK, scalar2=SIGN,
                                            op0=Alu.bitwise_and,
                                            op1=Alu.bitwise_xor)
                    nc.vector.tensor_scalar(out=d3[:], in0=d3[:],
                                            scalar1=1e-8, scalar2=None,
                                            op0=Alu.add)
                    nc.vector.reciprocal(d3[:], d3[:])
                    rsum = spool.tile([P, CP], f32, tag="rsum")
                    nc.vector.tensor_reduce(out=rsum[:], in_=d3[:], axis=X,
                                            op=Alu.add)
                    nc.vector.reciprocal(rsum[:], rsum[:])
                    w3 = spool.tile(sh3, f32, tag="w3")
                    nc.vector.tensor_tensor(
                        out=w3[:], in0=d3[:],
                        in1=rsum[:].unsqueeze(2).to_broadcast(sh3),
                        op=Alu.mult)

                    # ---- feature gather (one dma_gather per group) ----
                    wn3 = wrap_bounce("n3", n3_16[:], CP * 3, dpool, spool)
                    g = gpool.tile([P, CP, 3, D], f16, tag="g")
                    nc.gpsimd.dma_gather(
                        out_ap=g[:].rearrange("p c k f -> p (c k) f"),
                        in_ap=p1t[:],
                        idxs_ap=wn3[:].bitcast(i16),
                        num_idxs=CP * 3 * P,
                        num_idxs_reg=CP * 3 * P,
                        elem_size=D,
                        single_packet=False,
                    )

                    # ---- interpolate + transpose out ----
                    for cc in range(CP):
                        ci = gg * CP + cc
                        acc = opool.tile([P, D], f16, tag="acc")
                        nc.vector.tensor_scalar(
                            out=acc[:], in0=g[:, cc, 0, :],
                            scalar1=w3[:, cc, 0:1], scalar2=None,
                            op0=Alu.mult)
                        for k in (1, 2):
                            gm = opool.tile([P, D], f16, tag="gm")
                            nc.vector.tensor_scalar(
                                out=gm[:], in0=g[:, cc, k, :],
                                scalar1=w3[:, cc, k:k + 1], scalar2=None,
                                op0=Alu.mult)
                            nc.vector.tensor_tensor(out=acc[:], in0=acc[:],
                                                    in1=gm[:], op=Alu.add)

                        for dh in range(2):
                            pt = tppool.tile([P, P], f16, tag="tp")
                            nc.tensor.transpose(
                                pt[:], acc[:, dh * P:(dh + 1) * P], ident[:])
                            ot = opool.tile([P, P], f32, tag="ot")
                            nc.scalar.copy(ot[:], pt[:])
                            nc.sync.dma_start(
                                outT[dh * P:(dh + 1) * P, ci * P:(ci + 1) * P],
                                ot[:])

    nc.finalize()
    return nc


def _split2(x):
    """Split fp64 array into 2 bf16 terms h+l ~ x (residual ~2^-17|x|)."""
    bf = ml_dtypes.bfloat16
    h = x.astype(bf)
    r = x - h.astype(np.float64)
    l = r.astype(bf)
    return h, l


def _host_matrices(xyz2b, xyz1b):
    """Build the K=13 bf16 contraction matrices for one batch.

    negdist[s, n] = sum_k X2[k, s] * X1[k, n]
                  = 2 * x2_s . x1_n - |x2_s|^2 - |x1_n|^2   (+O(2^-16))
    """
    bf = ml_dtypes.bfloat16
    x2 = xyz2b.astype(np.float64)   # [3, S]
    x1 = xyz1b.astype(np.float64)   # [3, N]
    n2 = (x2 * x2).sum(axis=0)      # [S]
    n1 = (x1 * x1).sum(axis=0)      # [N]

    Srows, Nrows = [], []
    for c in range(3):
        h2, l2 = _split2(x2[c])
        h1, l1 = _split2(x1[c])
        th2 = (2.0 * h2.astype(np.float64)).astype(bf)
        tl2 = (2.0 * l2.astype(np.float64)).astype(bf)
        # products kept: hh hl lh  (ll and smaller dropped)
        for a, b_ in ((th2, h1), (th2, l1), (tl2, h1)):
            Srows.append(a)
            Nrows.append(b_)
    ones_s = np.ones(x2.shape[1], dtype=bf)
    ones_n = np.ones(x1.shape[1], dtype=bf)
    for t in _split2(-n2):
        Srows.append(t)
        Nrows.append(ones_n)
    for t in _split2(-n1):
        Srows.append(ones_s)
        Nrows.append(t)
    X2 = np.stack([np.asarray(r, dtype=bf) for r in Srows])   # [13, S]
    X1 = np.stack([np.asarray(r, dtype=bf) for r in Nrows])   # [13, N]
    return X2, X1, n2.astype(np.float32), n1.astype(np.float32)


def _prep_inputs(xyz1, xyz2, points1):
    f16 = np.float16
    xyz1 = np.asarray(xyz1, dtype=np.float32)
    xyz2 = np.asarray(xyz2, dtype=np.float32)
    points1 = np.asarray(points1, dtype=np.float32)
    in_maps = []
    for b in range(B):
        X2, X1, n2, n1 = _host_matrices(xyz2[b], xyz1[b])
        p1tb = np.ascontiguousarray(points1[b].T).astype(f16)  # [N, D] fp16
        # block table: row j holds points {j + 512*m}, each [x, y, z, 0]
        xb = np.zeros((NBLK, BPTS, 4), dtype=np.float32)
        pts = xyz1[b].T.reshape(BPTS, NBLK, 3)     # [m, j, 3]
        xb[:, :, 0:3] = pts.transpose(1, 0, 2)
        # per-query [x, y, z, n2], laid out [p, chunk, 4]
        xq = np.empty((P, NCHUNK, 4), dtype=np.float32)
        q = xyz2[b].T.reshape(NCHUNK, P, 3)        # [chunk, p, 3]
        xq[:, :, 0:3] = q.transpose(1, 0, 2)
        xq[:, :, 3] = n2.reshape(NCHUNK, P).T
        in_maps.append({
            "x2m": X2, "x1m": X1, "p1t": p1tb,
            "xblk": xb.reshape(NBLK, BPTS * 4), "x2n": xq,
        })
    return in_maps


def _get_compiled():
    global _COMPILED
    if _COMPILED is None:
        _COMPILED = _build_bass()
    return _COMPILED


def kernel(xyz1, xyz2, points1):
    from concourse.bass_utils import run_bass_kernel_spmd

    nc = _get_compiled()
    in_maps = _prep_inputs(xyz1, xyz2, points1)
    res = run_bass_kernel_spmd(nc, in_maps, core_ids=list(range(B)))
    return np.stack([r["outT"] for r in res.results]).astype(np.float32)


if __name__ == "__main__":
    rng = np.random.default_rng(0)
    xyz1 = rng.standard_normal((B, 3, N), dtype=np.float32)
    xyz2 = rng.standard_normal((B, 3, S), dtype=np.float32)
    p1 = rng.standard_normal((B, D, N), dtype=np.float32)
    out = kernel(xyz1, xyz2, p1)
    print("out", out.shape, out.dtype)
